# revision 21
# baseline (speedup 1.0000x reference)
"""Trainium2 Bass kernel for nn_MultiHeadAttention_61357902791348.

Sharding: 12 heads on 8 cores. Core pair (2p, 2p+1) owns heads {3p, 3p+1}
fully and splits head 3p+2's query rows (even core: rows [0,1600), odd:
[1600,3200)) -- balanced head/sequence-hybrid tensor parallelism with no
device collectives. Each core emits partial out-projection results; the
host sums the 8 partials and adds bo.

v3 rewrite vs v2 (452us -> target ~260us):
  * attention q-chunks are 1024 wide: exp runs one ACTIVATE per k-tile
    over a 2-bank PSUM score tile, amortizing the 352-cycle ACT fixed
    cost (ACT was the bottleneck engine of the attention phase)
  * projection blocks reordered K,V,Q and rope(k)/rms-stats/V-transposes
    emitted as filler between blocks: the PE stream stays dense through
    what used to be a fragmented half-clock (HAM-throttled) phase
  * q rms factors computed in row layout directly (ssq row -> sqrt ->
    reciprocal -> rows_all), deleting v2's column transpose + flatten
    machinery and its PSUM bank
  * rope(q) for chunk ci+1 is emitted inside chunk ci's score stream so
    score matmuls never wait on the DVE rope ops at a chunk boundary
  * softmax sum chains run on 1024-wide tiles split DVE/gpsimd; the two
    chains reduce into one PSUM row via two accumulating ones-matmuls
  * out-projection PSUM->SBUF copies (with the deferred 1/sum scale)
    split across DVE and gpsimd; exp keeps ACT saturated
"""

import numpy as np

B, N, D = 1, 3200, 1536
NH, HD = 12, 128
F, Hg, Wg = 8, 20, 20
EPS = 1e-6
NS = 1600          # shared-head query rows per core
NCH = D // 128     # 12 D-chunks
PW = 400           # projection moving tile width
RW = 512           # rope k tile width
CW = 1024          # attention q-chunk width
NQ = N + NS        # 4800 q tokens per core (own + shared)
NK = 2 * N         # 6400 k tokens per core (own + shared heads)

_CACHE = {}


def subtiles(total, width):
    return [(o, min(width, total - o)) for o in range(0, total, width)]


def _build():
    import concourse.bacc as bacc
    import concourse.mybir as mybir
    import concourse.tile as tile

    F32 = mybir.dt.float32
    F32R = mybir.dt.float32r
    BF16 = mybir.dt.bfloat16
    AF = mybir.ActivationFunctionType

    nc = bacc.Bacc("TRN2", target_bir_lowering=False, debug=False)

    xT = nc.dram_tensor("xT", [D, N], BF16, kind="ExternalInput")
    w6 = nc.dram_tensor("w6", [6, 128, NCH, HD], BF16, kind="ExternalInput")
    bias6 = nc.dram_tensor("bias6", [HD, 6], F32, kind="ExternalInput")
    cq = nc.dram_tensor("cq", [HD, N], BF16, kind="ExternalInput")
    sq = nc.dram_tensor("sq", [HD, N], BF16, kind="ExternalInput")
    ck = nc.dram_tensor("ck", [HD, N], BF16, kind="ExternalInput")
    sk = nc.dram_tensor("sk", [HD, N], BF16, kind="ExternalInput")
    rotm = nc.dram_tensor("rotm", [HD, HD], BF16, kind="ExternalInput")
    eye = nc.dram_tensor("eye", [HD, HD], BF16, kind="ExternalInput")
    ones2d = nc.dram_tensor("ones2d", [HD, HD], BF16, kind="ExternalInput")
    ones2df = nc.dram_tensor("ones2df", [1, HD], F32R, kind="ExternalInput")
    wo2 = nc.dram_tensor("wo2", [2, HD, D], BF16, kind="ExternalInput")
    y_own = nc.dram_tensor("y_own", [N, D], BF16, kind="ExternalOutput")
    y_sh = nc.dram_tensor("y_sh", [NS, D], BF16, kind="ExternalOutput")

    with tile.TileContext(nc) as tc:
        import contextlib

        stack = contextlib.ExitStack()
        with stack:
            persist = stack.enter_context(tc.tile_pool(name="persist", bufs=1))
            qT = persist.tile([128, NQ], BF16, tag="qT")
            kT = persist.tile([128, NK], BF16, tag="kT")
            vtok = persist.tile([128, 2, 25, HD], BF16, tag="vtok")
            aoT = persist.tile([128, NQ], BF16, tag="aoT")
            bias_sb = persist.tile([HD, 6], F32, tag="bias")
            nc.sync.dma_start(bias_sb[:], bias6[:])
            ones_sb = persist.tile([HD, HD], BF16, tag="ones")
            nc.sync.dma_start(ones_sb[:], ones2d[:])
            rot_sb = persist.tile([HD, HD], BF16, tag="rot")
            eye_sb = persist.tile([HD, HD], BF16, tag="eye")
            ones_f = persist.tile([1, HD], F32R, tag="onesf")
            nc.sync.dma_start(ones_f[:], ones2df[:])
            wo_sb = [persist.tile([HD, D], BF16, tag=f"wo{u}", name=f"wo_sb{u}")
                     for u in range(2)]
            tabs = {}
            for nm in ("cq", "sq", "ck", "sk"):
                tabs[nm] = persist.tile([128, N], BF16, tag=f"tab_{nm}",
                                        name=f"tab_{nm}")

            rsk_sb = persist.tile([128, 50], F32, tag="rsk")
            inv_sb = persist.tile([128, NQ // 128 + 1], F32, tag="inv")
            rows_all = persist.tile([1, NQ], F32R, tag="rows_all")
            bias_q1 = persist.tile([1, 1], F32, tag="bias_q1")
            nc.vector.memset(bias_q1[:], HD * EPS)
            bias_k = persist.tile([128, 1], F32, tag="bias_k")
            nc.vector.memset(bias_k[:], EPS)

            rtmp = stack.enter_context(tc.tile_pool(name="rtmp", bufs=3))
            rowtmp = stack.enter_context(tc.tile_pool(name="rowtmp", bufs=2))
            pools = {}

            def k_stats(o, w, uid):
                """kT[:, o:o+w] raw k -> rsk_sb columns (per-token rms
                reciprocal, exp scale layout: k tokens on partitions)."""
                rowp = pools["rowp"]
                src = kT[:, o:o + w]
                q2 = rtmp.tile([128, RW], BF16, tag="q2", name=f"kq2{uid}")
                nc.gpsimd.tensor_mul(q2[:, :w], src, src)
                ssq = rowp.tile([1, RW], F32, tag="row", name=f"kssq{uid}")
                nc.tensor.matmul(ssq[:, :w], ones_sb[:, 0:1], q2[:, :w],
                                 start=True, stop=True)
                srow = rowtmp.tile([1, RW], F32R, tag="srow", name=f"ksr{uid}")
                nc.scalar.activation(srow[:, :w], ssq[:, :w], AF.Copy)
                smt = rowp.tile([128, 8], F32, tag="row", name=f"ksmt{uid}")
                ncc = (w + 127) // 128
                for i in range(ncc):
                    lo = i * 128
                    ccw = min(128, w - lo)
                    nc.tensor.matmul(
                        smt[0:ccw, 2 * i:2 * i + 2],
                        srow[0:1, lo:lo + ccw],
                        ones_f[0:1, 0:2],
                        start=True, stop=True)
                sq_t = rtmp.tile([128, 8], F32, tag="sqt", name=f"ksq{uid}")
                nc.scalar.activation(sq_t[:, :ncc], smt[:, 0:2 * ncc:2],
                                     AF.Sqrt, bias=bias_k[:], scale=1.0 / HD)
                nc.vector.reciprocal(
                    rsk_sb[:, o // 128:o // 128 + ncc], sq_t[:, :ncc])

            def q_stats(o, w, uid):
                """qT[:, o:o+w] raw q -> rows_all row segment (rms recip
                with 1/sqrt(HD) folded, broadcast layout for rope)."""
                rowp = pools["rowp"]
                src = qT[:, o:o + w]
                q2 = rtmp.tile([128, RW], BF16, tag="q2", name=f"qq2{uid}")
                nc.gpsimd.tensor_mul(q2[:, :w], src, src)
                ssq = rowp.tile([1, RW], F32, tag="row", name=f"qssq{uid}")
                nc.tensor.matmul(ssq[:, :w], ones_sb[:, 0:1], q2[:, :w],
                                 start=True, stop=True)
                srow = rowtmp.tile([1, RW], F32, tag="srow", name=f"qsr{uid}")
                nc.scalar.activation(srow[:, :w], ssq[:, :w], AF.Sqrt,
                                     bias=bias_q1[:], scale=1.0)
                # f32r keeps 13 mantissa bits -- ample for an rms scale
                with nc.allow_low_precision(reason="f32r rope scale row"):
                    nc.vector.reciprocal(rows_all[0:1, o:o + w], srow[:, :w])

            def k_apply(o, tok, w, uid):
                """In-place rope on kT[:, o:o+w] (tokens tok..tok+w)."""
                src = kT[:, o:o + w]
                # rot shares the rowp pool+tag so back-to-back k_applys
                # don't serialize on a single psum slot
                rot = pools["rowp"].tile([128, RW], F32, tag="row",
                                         name=f"krt{uid}")
                nc.tensor.matmul(rot[:, :w], rot_sb[:], src,
                                 start=True, stop=True)
                m1 = rtmp.tile([128, RW], BF16, tag="m1", name=f"km1{uid}")
                nc.gpsimd.tensor_mul(m1[:, :w], src,
                                     tabs["ck"][:, tok:tok + w])
                m2 = rtmp.tile([128, RW], BF16, tag="m2", name=f"km2{uid}")
                nc.vector.tensor_mul(m2[:, :w], rot[:, :w],
                                     tabs["sk"][:, tok:tok + w])
                nc.gpsimd.tensor_add(src, m1[:, :w], m2[:, :w])

            def v_transpose(vT, h, g, vtp):
                tpg = vtp.tile([128, 5, HD], BF16, tag="tp",
                               name=f"tp{h}_{g}")
                for k5 in range(5):
                    jt = g * 5 + k5
                    nc.tensor.transpose(
                        tpg[:, k5, :],
                        vT[:, h * N + jt * 128:h * N + (jt + 1) * 128],
                        eye_sb[:])
                nc.scalar.activation(vtok[:, h, g * 5:(g + 1) * 5, :],
                                     tpg[:], AF.Copy)

            # ---------------- projection phase ----------------
            # block order per half: k_own, k_sh, v_own, v_sh, q_own(, q_sh)
            # fillers keyed by global block index, emitted after that
            # block's matmuls so rope/stats overlap later projections.
            ko = subtiles(N, RW)          # 7 tiles per head segment

            with tc.tile_pool(name="vt", bufs=1) as vt_pool:
                vT = vt_pool.tile([128, NK], BF16, tag="vT")
                with tc.tile_pool(name="xt", bufs=1) as xt_pool, \
                     tc.tile_pool(name="wld", bufs=3) as w_pool, \
                     tc.tile_pool(name="pp", bufs=4, space="PSUM") as pp, \
                     tc.tile_pool(name="rowp", bufs=2, space="PSUM") as rowp, \
                     tc.tile_pool(name="vtp", bufs=2, space="PSUM") as vtp:
                    pools["rowp"] = rowp

                    def fillers_for(idx):
                        if idx == 2:    # after v_own h0: k raw h0 ready
                            for i in (0, 1, 2):
                                k_stats(ko[i][0], ko[i][1], f"o{i}")
                        elif idx == 3:
                            for i in (0, 1, 2):
                                k_stats(N + ko[i][0], ko[i][1], f"s{i}")
                            for i in (0, 1):
                                k_apply(ko[i][0], ko[i][0], ko[i][1], f"o{i}")
                        elif idx == 4:
                            k_apply(ko[2][0], ko[2][0], ko[2][1], "o2")
                            for i in (0, 1):
                                k_apply(N + ko[i][0], ko[i][0], ko[i][1],
                                        f"s{i}")
                        elif idx == 5:
                            k_apply(N + ko[2][0], ko[2][0], ko[2][1], "s2")
                            for i in (0, 1, 2):
                                q_stats(ko[i][0], ko[i][1], f"o{i}")
                            for (ol, w) in subtiles(NS, RW):
                                q_stats(N + ol, w, f"s{ol}")
                        elif idx == 6:  # after k_own h1: all own k raw
                            for i in (3, 4, 5, 6):
                                k_stats(ko[i][0], ko[i][1], f"o{i}")
                        elif idx == 7:
                            for i in (3, 4):
                                k_apply(ko[i][0], ko[i][0], ko[i][1], f"o{i}")
                            for i in (3, 4, 5, 6):
                                k_stats(N + ko[i][0], ko[i][1], f"s{i}")
                        elif idx == 8:  # after q_own h1 (moved before v)
                            for i in (5, 6):
                                k_apply(ko[i][0], ko[i][0], ko[i][1], f"o{i}")
                            for i in (3, 4):
                                k_apply(N + ko[i][0], ko[i][0], ko[i][1],
                                        f"s{i}")
                            for i in (3, 4, 5, 6):
                                q_stats(ko[i][0], ko[i][1], f"o{i}")
                        elif idx == 9:  # after v_own h1
                            for i in (5, 6):
                                k_apply(N + ko[i][0], ko[i][0], ko[i][1],
                                        f"s{i}")
                            for g in range(5):
                                v_transpose(vT, 0, g, vtp)
                        elif idx == 10:  # after v_sh h1
                            for g in range(5):
                                v_transpose(vT, 1, g, vtp)

                    wt_next = None
                    blk_idx = 0
                    for half in range(2):
                        h0 = half * 1600
                        order = ([2, 3, 4, 5, 0, 1] if half == 0
                                 else [2, 3, 0, 4, 5])
                        if half == 0:
                            wt_next = w_pool.tile([128, NCH, HD], BF16,
                                                  tag="w", name="wt_first")
                            nc.sync.dma_start(wt_next[:], w6[order[0], :, :, :])
                        xts = []
                        for c in range(NCH):
                            xt = xt_pool.tile([128, 1600], BF16,
                                              tag=f"xt{c}", name=f"xt{half}_{c}")
                            nc.sync.dma_start(
                                xt[:], xT[c * 128:(c + 1) * 128, h0:h0 + 1600])
                            xts.append(xt)
                        if half == 0:
                            # k tables + rot/eye land during h0 compute,
                            # before the first k_apply/v_transpose fillers
                            nc.sync.dma_start(tabs["ck"][:], ck[:])
                            nc.sync.dma_start(tabs["sk"][:], sk[:])
                            nc.sync.dma_start(rot_sb[:], rotm[:])
                            nc.sync.dma_start(eye_sb[:], eye[:])
                        else:
                            nc.sync.dma_start(tabs["cq"][:], cq[:])
                            nc.sync.dma_start(tabs["sq"][:], sq[:])
                            for u in range(2):
                                nc.sync.dma_start(wo_sb[u][:], wo2[u, :, :])
                        for b in order:
                            if b == 0:
                                dst, d0 = qT, h0
                            elif b == 1:
                                dst, d0 = qT, N + h0
                            elif b in (2, 3):
                                dst, d0 = kT, (b - 2) * N + h0
                            else:
                                dst, d0 = vT, (b - 4) * N + h0
                            if wt_next is not None:
                                wt_all, wt_next = wt_next, None
                            else:
                                wt_all = w_pool.tile([128, NCH, HD], BF16,
                                                     tag="w",
                                                     name=f"wt{half}_{b}")
                                nc.sync.dma_start(wt_all[:], w6[b, :, :, :])
                            tiles4 = subtiles(1600, PW)
                            pss = [pp.tile([128, PW], F32, tag="pp",
                                           name=f"pp{half}_{b}_{oi}")
                                   for oi in range(len(tiles4))]
                            for c in range(NCH):
                                for oi, (o, w) in enumerate(tiles4):
                                    nc.tensor.matmul(
                                        pss[oi][:, :w], wt_all[:, c, :],
                                        xts[c][:, o:o + w],
                                        start=(c == 0), stop=(c == NCH - 1),
                                    )
                                    if c == NCH - 1:
                                        # free the psum slot asap (pp bufs=4)
                                        nc.vector.tensor_scalar_add(
                                            dst[:, d0 + o:d0 + o + w],
                                            pss[oi][:, :w],
                                            bias_sb[:, b:b + 1],
                                        )
                            fillers_for(blk_idx)
                            blk_idx += 1

            # ---------------- attention phase ----------------
            # chunks: (unit, gco, tok, cw)
            chunks = []
            for (ol, w) in subtiles(N, CW):
                chunks.append((0, ol, ol, w))
            for (ol, w) in subtiles(NS, CW):
                chunks.append((1, N + ol, ol, w))
            NCHUNK = len(chunks)

            # psum pools: sc 2x2 + ao 2 + op 2 = 8 banks
            with tc.tile_pool(name="scp", bufs=2, space="PSUM") as scp, \
                 tc.tile_pool(name="aop", bufs=2, space="PSUM") as aop, \
                 tc.tile_pool(name="opp", bufs=2, space="PSUM") as opp, \
                 tc.tile_pool(name="expp", bufs=6) as expp, \
                 tc.tile_pool(name="accp", bufs=2) as accp, \
                 tc.tile_pool(name="qrt", bufs=2) as qrt, \
                 tc.tile_pool(name="yout", bufs=3) as yout:

                state = {}

                def rope_q(ci):
                    unit, gco, tok, cw = chunks[ci]
                    src = qT[:, gco:gco + cw]
                    rot = scp.tile([128, CW], F32, tag="sc",
                                   name=f"qrot{ci}")
                    for (off, hw) in subtiles(cw, 512):
                        nc.tensor.matmul(rot[:, off:off + hw], rot_sb[:],
                                         qT[:, gco + off:gco + off + hw],
                                         start=True, stop=True)
                    bcp = scp.tile([128, CW], F32, tag="sc",
                                   name=f"qbcp{ci}")
                    for i in range((cw + 127) // 128):
                        lo = i * 128
                        ccw = min(128, cw - lo)
                        nc.tensor.matmul(
                            bcp[:, lo:lo + ccw],
                            ones_f[0:1, :],
                            rows_all[0:1, gco + lo:gco + lo + ccw],
                            start=True, stop=True)
                    m1 = qrt.tile([128, CW], BF16, tag="m1", name=f"qm1{ci}")
                    nc.gpsimd.tensor_mul(m1[:, :cw], src,
                                         tabs["cq"][:, tok:tok + cw])
                    m2 = qrt.tile([128, CW], BF16, tag="m2", name=f"qm2{ci}")
                    nc.vector.tensor_mul(m2[:, :cw], rot[:, :cw],
                                         tabs["sq"][:, tok:tok + cw])
                    qr = qrt.tile([128, CW], BF16, tag="qr", name=f"qqr{ci}")
                    nc.gpsimd.tensor_add(qr[:, :cw], m1[:, :cw], m2[:, :cw])
                    nc.vector.tensor_mul(src, qr[:, :cw], bcp[:, :cw])

                def tail(ci):
                    """softmax denominators for chunk ci -> inv_sb cols."""
                    acc = state.pop(ci)
                    unit, gco, tok, cw = chunks[ci]
                    g0 = gco // 128
                    hs = subtiles(cw, 512)
                    sms = []
                    for (off, hw) in hs:
                        sm = opp.tile([128, 512], F32, tag="op",
                                      name=f"sm{ci}_{off}")
                        nc.tensor.matmul(sm[0:1, :hw], ones_sb[:, 0:1],
                                         acc[:, off:off + hw],
                                         start=True, stop=True)
                        sms.append(sm)
                    smrow = rowtmp.tile([1, CW], F32R, tag="smrow",
                                        name=f"smr{ci}")
                    for (off, hw), sm in zip(hs, sms):
                        nc.vector.tensor_copy(smrow[:, off:off + hw],
                                              sm[0:1, :hw])
                    smt = opp.tile([128, 512], F32, tag="op",
                                   name=f"smt{ci}")
                    ncc = (cw + 127) // 128
                    for i in range(ncc):
                        lo = i * 128
                        ccw = min(128, cw - lo)
                        nc.tensor.matmul(
                            smt[0:ccw, 2 * i:2 * i + 2],
                            smrow[0:1, lo:lo + ccw],
                            ones_f[0:1, 0:2],
                            start=True, stop=True)
                    nc.vector.reciprocal(inv_sb[:, g0:g0 + ncc],
                                         smt[:, 0:2 * ncc:2])

                cpat_ctr = [0]

                def outproj_tile(ci, ti, copy_eng=None):
                    """one 128-token tile of the out-projection for chunk
                    ci: 3 matmuls + 3 scaled psum->sbuf copies + dma."""
                    if copy_eng is None:
                        copy_eng = CPAT[cpat_ctr[0] % len(CPAT)]
                        cpat_ctr[0] += 1
                    unit, gco, tok, cw = chunks[ci]
                    ydst = y_own if unit == 0 else y_sh
                    it = ti * 128
                    iw = min(128, cw - it)
                    git = gco + it
                    gidx = git // 128
                    yt = yout.tile([128, D], BF16, tag="yt",
                                   name=f"yt{ci}_{ti}")
                    for ct3 in range(3):
                        op = opp.tile([128, 512], F32, tag="op",
                                      name=f"op{ci}_{ti}_{ct3}")
                        nc.tensor.matmul(
                            op[0:iw, :], aoT[:, git:git + iw],
                            wo_sb[unit][:, ct3 * 512:(ct3 + 1) * 512],
                            start=True, stop=True)
                        eng = copy_eng[ct3]
                        if eng == "d":
                            nc.vector.tensor_scalar_mul(
                                yt[0:iw, ct3 * 512:(ct3 + 1) * 512],
                                op[0:iw, :], inv_sb[0:iw, gidx:gidx + 1])
                        elif eng == "g":
                            nc.gpsimd.tensor_scalar_mul(
                                yt[0:iw, ct3 * 512:(ct3 + 1) * 512],
                                op[0:iw, :], inv_sb[0:iw, gidx:gidx + 1])
                        else:
                            nc.scalar.activation(
                                yt[0:iw, ct3 * 512:(ct3 + 1) * 512],
                                op[0:iw, :], AF.Copy,
                                scale=inv_sb[0:iw, gidx:gidx + 1])
                    nc.sync.dma_start(ydst[tok + it:tok + it + iw, :],
                                      yt[0:iw, :])

                # out-copy engines: only DVE and ACT can read PSUM;
                # ACT is exp-saturated mid-stream so the main loop uses DVE
                CPAT = ["ddd"]

                op_pend = []

                def attention(ci, prev_ci, next_ci):
                    """score/exp/AV stream for chunk ci. AV trails exp by
                    one k-tile so the PE never waits on ACT; the previous
                    chunk's softmax tail + pending out-projection tiles
                    and the next chunk's rope are interleaved."""
                    unit, gco, tok, cw = chunks[ci]
                    head = unit
                    hs = subtiles(cw, 512)
                    aos = [aop.tile([128, 512], F32, tag="ao",
                                    name=f"ao{ci}_{off}") for (off, hw) in hs]
                    acc = accp.tile([128, CW], BF16, tag="acc",
                                    name=f"acc{ci}")
                    # out-proj consumption capacity scales with chunk width
                    cap = max(1, cw * 11 // 1024)
                    op_jts = set([3, 5, 7, 9, 11, 13, 15, 17, 19, 21, 23][:cap])
                    prev = None

                    def av(ex, jt):
                        for oi, (off, hw) in enumerate(hs):
                            nc.tensor.matmul(
                                aos[oi][:, :hw], vtok[:, head, jt, :],
                                ex[:, off:off + hw],
                                start=(jt == 0), stop=(jt == 24))

                    for jt in range(25):
                        gjt = head * 25 + jt
                        sc = scp.tile([128, CW], F32, tag="sc",
                                      name=f"sc{ci}_{jt}")
                        for (off, hw) in hs:
                            nc.tensor.matmul(
                                sc[:, off:off + hw],
                                kT[:, gjt * 128:(gjt + 1) * 128],
                                qT[:, gco + off:gco + off + hw],
                                start=True, stop=True)
                        ex = expp.tile([128, CW], BF16, tag="ex",
                                       name=f"ex{ci}_{jt}")
                        nc.scalar.activation(ex[:, :cw], sc[:, :cw], AF.Exp,
                                             scale=rsk_sb[:, gjt:gjt + 1])
                        if prev is not None:
                            av(*prev)
                        prev = (ex, jt)
                        # softmax sums accumulate on the DMA engines
                        # (SBUF->SBUF with fp32 add), freeing DVE/gpsimd
                        if jt == 0:
                            nc.gpsimd.dma_start(acc[:, :cw], ex[:, :cw])
                        else:
                            nc.gpsimd.dma_start(
                                acc[:, :cw], ex[:, :cw],
                                accum_op=mybir.AluOpType.add)
                        if jt == 2 and prev_ci is not None:
                            tail(prev_ci)
                        if jt == 4 and next_ci is not None:
                            rope_q(next_ci)
                        if jt in op_jts and op_pend:
                            outproj_tile(*op_pend.pop(0))
                    av(*prev)
                    # copy ao psum -> aoT (frees the ao banks for ci+1)
                    for oi, (off, hw) in enumerate(hs):
                        nc.vector.tensor_copy(
                            aoT[:, gco + off:gco + off + hw],
                            aos[oi][:, :hw])
                    state[ci] = acc
                    for ti in range((cw + 127) // 128):
                        op_pend.append((ci, ti))

                # the narrow own-head chunk goes first so it never has to
                # absorb a full chunk's out-projection work; the 576-wide
                # shared chunk sits mid-sequence so wide chunks drain its
                # backlog before the epilogue
                corder = [3, 0, 1, 5, 2, 4]
                rope_q(corder[0])
                for oi_ in range(len(corder)):
                    attention(corder[oi_],
                              corder[oi_ - 1] if oi_ > 0 else None,
                              corder[oi_ + 1] if oi_ + 1 < len(corder) else None)
                tail(corder[-1])
                # epilogue drain: exp is done, so ACT helps with copies
                EPAT = ["ada", "dad"]
                ei = 0
                while op_pend:
                    outproj_tile(*op_pend.pop(0), copy_eng=EPAT[ei % 2])
                    ei += 1

    nc.compile()
    return nc


def _get_nc():
    if "nc" not in _CACHE:
        _CACHE["nc"] = _build()
    return _CACHE["nc"]


def _host_prep(inputs):
    import ml_dtypes

    bf16 = ml_dtypes.bfloat16
    x = np.asarray(inputs["x"], np.float32)[0]          # [N, D]
    Wq = np.asarray(inputs["Wq"], np.float32)
    Wk = np.asarray(inputs["Wk"], np.float32)
    Wv = np.asarray(inputs["Wv"], np.float32)
    Wo = np.asarray(inputs["Wo"], np.float32)
    bq = np.asarray(inputs["bq"], np.float32)
    bk = np.asarray(inputs["bk"], np.float32)
    bv = np.asarray(inputs["bv"], np.float32)
    qs = np.asarray(inputs["q_scale"], np.float32)
    ks = np.asarray(inputs["k_scale"], np.float32)
    ft = np.asarray(inputs["freqs_t"], np.float32)
    fh = np.asarray(inputs["freqs_h"], np.float32)
    fw = np.asarray(inputs["freqs_w"], np.float32)

    cos = np.zeros((N, HD // 2), np.float32)
    sin = np.zeros((N, HD // 2), np.float32)
    idx = np.arange(N)
    f_idx, h_idx, w_idx = idx // (Hg * Wg), (idx // Wg) % Hg, idx % Wg
    cos[:, 0:22], sin[:, 0:22] = ft[f_idx, :, 0], ft[f_idx, :, 1]
    cos[:, 22:43], sin[:, 22:43] = fh[h_idx, :, 0], fh[h_idx, :, 1]
    cos[:, 43:64], sin[:, 43:64] = fw[w_idx, :, 0], fw[w_idx, :, 1]
    C = np.repeat(cos, 2, axis=1).T.copy()               # [128, N]
    S = np.repeat(sin, 2, axis=1).T.copy()
    qs_sw = qs.reshape(64, 2)[:, ::-1].reshape(128)
    ks_sw = ks.reshape(64, 2)[:, ::-1].reshape(128)
    Cq, Sq = C * qs[:, None], S * qs_sw[:, None]
    Ck, Sk = C * ks[:, None], S * ks_sw[:, None]

    rotm = np.zeros((128, 128), np.float32)
    pr = np.arange(64)
    rotm[2 * pr + 1, 2 * pr] = -1.0
    rotm[2 * pr, 2 * pr + 1] = 1.0
    eye = np.eye(128, dtype=np.float32)
    ones2d = np.ones((128, 128), np.float32)

    xT = np.ascontiguousarray(x.T)                       # [D, N]
    perm_swap = np.concatenate([np.arange(1600, N), np.arange(0, 1600)])

    in_maps = []
    for core in range(8):
        pair, parity = core // 2, core % 2
        own, sh = 3 * pair + parity, 3 * pair + 2
        if parity == 0:
            xTc, Cqc, Sqc, Ckc, Skc = xT, Cq, Sq, Ck, Sk
        else:
            xTc = np.ascontiguousarray(xT[:, perm_swap])
            Cqc = np.ascontiguousarray(Cq[:, perm_swap])
            Sqc = np.ascontiguousarray(Sq[:, perm_swap])
            Ckc = np.ascontiguousarray(Ck[:, perm_swap])
            Skc = np.ascontiguousarray(Sk[:, perm_swap])
        w6c = np.stack([
            Wq[:, own * HD:(own + 1) * HD], Wq[:, sh * HD:(sh + 1) * HD],
            Wk[:, own * HD:(own + 1) * HD], Wk[:, sh * HD:(sh + 1) * HD],
            Wv[:, own * HD:(own + 1) * HD], Wv[:, sh * HD:(sh + 1) * HD],
        ])
        # [6, D, HD] -> [6, 128, NCH, HD] so each block loads in ONE dma
        w6c = w6c.reshape(6, NCH, 128, HD).transpose(0, 2, 1, 3)
        bias6 = np.stack([
            bq[own * HD:(own + 1) * HD], bq[sh * HD:(sh + 1) * HD],
            bk[own * HD:(own + 1) * HD], bk[sh * HD:(sh + 1) * HD],
            bv[own * HD:(own + 1) * HD], bv[sh * HD:(sh + 1) * HD],
        ], axis=1)
        wo2 = np.stack([
            Wo[own * HD:(own + 1) * HD, :], Wo[sh * HD:(sh + 1) * HD, :],
        ])
        in_maps.append({
            "xT": xTc.astype(bf16), "w6": np.ascontiguousarray(w6c).astype(bf16),
            "bias6": np.ascontiguousarray(bias6),
            "cq": Cqc.astype(bf16), "sq": Sqc.astype(bf16),
            "ck": Ckc.astype(bf16), "sk": Skc.astype(bf16),
            "rotm": rotm.astype(bf16), "eye": eye.astype(bf16),
            "ones2d": ones2d.astype(bf16),
            "ones2df": np.ones((1, 128), np.float32),
            "wo2": np.ascontiguousarray(wo2).astype(bf16),
        })
    return in_maps, perm_swap


def _gather(results, perm_swap, bo):
    inv_swap = perm_swap  # swapping halves is its own inverse
    y = np.zeros((N, D), np.float32)
    for core in range(8):
        parity = core % 2
        yo = np.asarray(results[core]["y_own"], np.float32)
        ysh = np.asarray(results[core]["y_sh"], np.float32)
        if parity == 0:
            y += yo
            y[0:1600] += ysh
        else:
            y += yo[inv_swap]
            y[1600:3200] += ysh
    y += bo[None, :]
    return y[None]


def run_internal(inputs, trace=False, **kw):
    from concourse.bass_utils import run_bass_kernel_spmd

    nc = _get_nc()
    in_maps, perm_swap = _host_prep(inputs)
    res = run_bass_kernel_spmd(
        nc, in_maps, core_ids=list(range(8)), trace=trace, **kw
    )
    bo = np.asarray(inputs["bo"], np.float32)
    y = _gather(res.results, perm_swap, bo)
    return y, res


def kernel(**inputs):
    y, _ = run_internal(inputs, trace=False)
    return y


# revision 22
# speedup vs baseline: 1.4355x; 1.4355x over previous
"""Trainium2 Bass kernel for nn_MultiHeadAttention_61357902791348.

Sharding: 12 heads on 8 cores. Core pair (2p, 2p+1) owns heads {3p, 3p+1}
fully and splits head 3p+2's query rows (even core: rows [0,1600), odd:
[1600,3200)) -- balanced head/sequence-hybrid tensor parallelism with no
device collectives. Each core emits partial out-projection results; the
host sums the 8 partials and adds bo.

v3 rewrite vs v2 (452us -> target ~260us):
  * attention q-chunks are 1024 wide: exp runs one ACTIVATE per k-tile
    over a 2-bank PSUM score tile, amortizing the 352-cycle ACT fixed
    cost (ACT was the bottleneck engine of the attention phase)
  * projection blocks reordered K,V,Q and rope(k)/rms-stats/V-transposes
    emitted as filler between blocks: the PE stream stays dense through
    what used to be a fragmented half-clock (HAM-throttled) phase
  * q rms factors computed in row layout directly (ssq row -> sqrt ->
    reciprocal -> rows_all), deleting v2's column transpose + flatten
    machinery and its PSUM bank
  * rope(q) for chunk ci+1 is emitted inside chunk ci's score stream so
    score matmuls never wait on the DVE rope ops at a chunk boundary
  * softmax sum chains run on 1024-wide tiles split DVE/gpsimd; the two
    chains reduce into one PSUM row via two accumulating ones-matmuls
  * out-projection PSUM->SBUF copies (with the deferred 1/sum scale)
    split across DVE and gpsimd; exp keeps ACT saturated
"""

import numpy as np

B, N, D = 1, 3200, 1536
NH, HD = 12, 128
F, Hg, Wg = 8, 20, 20
EPS = 1e-6
NS = 1600          # shared-head query rows per core
NCH = D // 128     # 12 D-chunks
PW = 400           # projection moving tile width
RW = 512           # rope k tile width
CW = 1024          # attention q-chunk width
NQ = N + NS        # 4800 q tokens per core (own + shared)
NK = 2 * N         # 6400 k tokens per core (own + shared heads)

_CACHE = {}


def subtiles(total, width):
    return [(o, min(width, total - o)) for o in range(0, total, width)]


def _build():
    import concourse.bacc as bacc
    import concourse.mybir as mybir
    import concourse.tile as tile

    F32 = mybir.dt.float32
    F32R = mybir.dt.float32r
    BF16 = mybir.dt.bfloat16
    AF = mybir.ActivationFunctionType

    nc = bacc.Bacc("TRN2", target_bir_lowering=False, debug=False)

    xT = nc.dram_tensor("xT", [D, N], BF16, kind="ExternalInput")
    w6 = nc.dram_tensor("w6", [6, 128, NCH, HD], BF16, kind="ExternalInput")
    bias6 = nc.dram_tensor("bias6", [HD, 6], F32, kind="ExternalInput")
    cq = nc.dram_tensor("cq", [HD, N], BF16, kind="ExternalInput")
    sq = nc.dram_tensor("sq", [HD, N], BF16, kind="ExternalInput")
    ck = nc.dram_tensor("ck", [HD, N], BF16, kind="ExternalInput")
    sk = nc.dram_tensor("sk", [HD, N], BF16, kind="ExternalInput")
    rotm = nc.dram_tensor("rotm", [HD, HD], BF16, kind="ExternalInput")
    eye = nc.dram_tensor("eye", [HD, HD], BF16, kind="ExternalInput")
    ones2d = nc.dram_tensor("ones2d", [HD, HD], BF16, kind="ExternalInput")
    ones2df = nc.dram_tensor("ones2df", [1, HD], F32R, kind="ExternalInput")
    wo2 = nc.dram_tensor("wo2", [2, HD, D], BF16, kind="ExternalInput")
    y_own = nc.dram_tensor("y_own", [N, D], BF16, kind="ExternalOutput")
    y_sh = nc.dram_tensor("y_sh", [NS, D], BF16, kind="ExternalOutput")

    with tile.TileContext(nc) as tc:
        import contextlib

        stack = contextlib.ExitStack()
        with stack:
            persist = stack.enter_context(tc.tile_pool(name="persist", bufs=1))
            qT = persist.tile([128, NQ], BF16, tag="qT")
            kT = persist.tile([128, NK], BF16, tag="kT")
            vtok = persist.tile([128, 2, 25, HD], BF16, tag="vtok")
            aoT = persist.tile([128, NQ], BF16, tag="aoT")
            bias_sb = persist.tile([HD, 6], F32, tag="bias")
            nc.sync.dma_start(bias_sb[:], bias6[:])
            ones_sb = persist.tile([HD, HD], BF16, tag="ones")
            nc.sync.dma_start(ones_sb[:], ones2d[:])
            rot_sb = persist.tile([HD, HD], BF16, tag="rot")
            eye_sb = persist.tile([HD, HD], BF16, tag="eye")
            ones_f = persist.tile([1, HD], F32R, tag="onesf")
            nc.sync.dma_start(ones_f[:], ones2df[:])
            wo_sb = [persist.tile([HD, D], BF16, tag=f"wo{u}", name=f"wo_sb{u}")
                     for u in range(2)]
            tabs = {}
            for nm in ("cq", "sq", "ck", "sk"):
                tabs[nm] = persist.tile([128, N], BF16, tag=f"tab_{nm}",
                                        name=f"tab_{nm}")

            rsk_sb = persist.tile([128, 50], F32, tag="rsk")
            inv_sb = persist.tile([128, NQ // 128 + 1], F32, tag="inv")
            rows_all = persist.tile([1, NQ], F32R, tag="rows_all")
            bias_q1 = persist.tile([1, 1], F32, tag="bias_q1")
            nc.vector.memset(bias_q1[:], HD * EPS)
            bias_k = persist.tile([128, 1], F32, tag="bias_k")
            nc.vector.memset(bias_k[:], EPS)

            rtmp = stack.enter_context(tc.tile_pool(name="rtmp", bufs=3))
            rowtmp = stack.enter_context(tc.tile_pool(name="rowtmp", bufs=2))
            pools = {}

            def k_stats(o, w, uid):
                """kT[:, o:o+w] raw k -> rsk_sb columns (per-token rms
                reciprocal, exp scale layout: k tokens on partitions)."""
                rowp = pools["rowp"]
                src = kT[:, o:o + w]
                q2 = rtmp.tile([128, RW], BF16, tag="q2", name=f"kq2{uid}")
                nc.gpsimd.tensor_mul(q2[:, :w], src, src)
                ssq = rowp.tile([1, RW], F32, tag="row", name=f"kssq{uid}")
                nc.tensor.matmul(ssq[:, :w], ones_sb[:, 0:1], q2[:, :w],
                                 start=True, stop=True)
                srow = rowtmp.tile([1, RW], F32R, tag="srow", name=f"ksr{uid}")
                nc.scalar.activation(srow[:, :w], ssq[:, :w], AF.Copy)
                smt = rowp.tile([128, 8], F32, tag="row", name=f"ksmt{uid}")
                ncc = (w + 127) // 128
                for i in range(ncc):
                    lo = i * 128
                    ccw = min(128, w - lo)
                    nc.tensor.matmul(
                        smt[0:ccw, 2 * i:2 * i + 2],
                        srow[0:1, lo:lo + ccw],
                        ones_f[0:1, 0:2],
                        start=True, stop=True)
                sq_t = rtmp.tile([128, 8], F32, tag="sqt", name=f"ksq{uid}")
                nc.scalar.activation(sq_t[:, :ncc], smt[:, 0:2 * ncc:2],
                                     AF.Sqrt, bias=bias_k[:], scale=1.0 / HD)
                nc.vector.reciprocal(
                    rsk_sb[:, o // 128:o // 128 + ncc], sq_t[:, :ncc])

            def q_stats(o, w, uid):
                """qT[:, o:o+w] raw q -> rows_all row segment (rms recip
                with 1/sqrt(HD) folded, broadcast layout for rope)."""
                rowp = pools["rowp"]
                src = qT[:, o:o + w]
                q2 = rtmp.tile([128, RW], BF16, tag="q2", name=f"qq2{uid}")
                nc.gpsimd.tensor_mul(q2[:, :w], src, src)
                ssq = rowp.tile([1, RW], F32, tag="row", name=f"qssq{uid}")
                nc.tensor.matmul(ssq[:, :w], ones_sb[:, 0:1], q2[:, :w],
                                 start=True, stop=True)
                srow = rowtmp.tile([1, RW], F32, tag="srow", name=f"qsr{uid}")
                nc.scalar.activation(srow[:, :w], ssq[:, :w], AF.Sqrt,
                                     bias=bias_q1[:], scale=1.0)
                # f32r keeps 13 mantissa bits -- ample for an rms scale
                with nc.allow_low_precision(reason="f32r rope scale row"):
                    nc.vector.reciprocal(rows_all[0:1, o:o + w], srow[:, :w])

            def k_apply(o, tok, w, uid):
                """In-place rope on kT[:, o:o+w] (tokens tok..tok+w)."""
                src = kT[:, o:o + w]
                # rot shares the rowp pool+tag so back-to-back k_applys
                # don't serialize on a single psum slot
                rot = pools["rowp"].tile([128, RW], F32, tag="row",
                                         name=f"krt{uid}")
                nc.tensor.matmul(rot[:, :w], rot_sb[:], src,
                                 start=True, stop=True)
                m1 = rtmp.tile([128, RW], BF16, tag="m1", name=f"km1{uid}")
                nc.gpsimd.tensor_mul(m1[:, :w], src,
                                     tabs["ck"][:, tok:tok + w])
                m2 = rtmp.tile([128, RW], BF16, tag="m2", name=f"km2{uid}")
                nc.vector.tensor_mul(m2[:, :w], rot[:, :w],
                                     tabs["sk"][:, tok:tok + w])
                nc.gpsimd.tensor_add(src, m1[:, :w], m2[:, :w])

            def v_transpose(vT, h, g, vtp):
                tpg = vtp.tile([128, 5, HD], BF16, tag="tp",
                               name=f"tp{h}_{g}")
                for k5 in range(5):
                    jt = g * 5 + k5
                    nc.tensor.transpose(
                        tpg[:, k5, :],
                        vT[:, h * N + jt * 128:h * N + (jt + 1) * 128],
                        eye_sb[:])
                nc.scalar.activation(vtok[:, h, g * 5:(g + 1) * 5, :],
                                     tpg[:], AF.Copy)

            # ---------------- projection phase ----------------
            # block order per half: k_own, k_sh, v_own, v_sh, q_own(, q_sh)
            # fillers keyed by global block index, emitted after that
            # block's matmuls so rope/stats overlap later projections.
            ko = subtiles(N, RW)          # 7 tiles per head segment

            with tc.tile_pool(name="vt", bufs=1) as vt_pool:
                vT = vt_pool.tile([128, NK], BF16, tag="vT")
                with tc.tile_pool(name="xt", bufs=1) as xt_pool, \
                     tc.tile_pool(name="wld", bufs=3) as w_pool, \
                     tc.tile_pool(name="pp", bufs=4, space="PSUM") as pp, \
                     tc.tile_pool(name="rowp", bufs=2, space="PSUM") as rowp, \
                     tc.tile_pool(name="vtp", bufs=2, space="PSUM") as vtp:
                    pools["rowp"] = rowp

                    def fillers_for(idx):
                        if idx == 2:    # after v_own h0: k raw h0 ready
                            for i in (0, 1, 2):
                                k_stats(ko[i][0], ko[i][1], f"o{i}")
                        elif idx == 3:
                            for i in (0, 1, 2):
                                k_stats(N + ko[i][0], ko[i][1], f"s{i}")
                            for i in (0, 1):
                                k_apply(ko[i][0], ko[i][0], ko[i][1], f"o{i}")
                        elif idx == 4:
                            k_apply(ko[2][0], ko[2][0], ko[2][1], "o2")
                            for i in (0, 1):
                                k_apply(N + ko[i][0], ko[i][0], ko[i][1],
                                        f"s{i}")
                        elif idx == 5:
                            k_apply(N + ko[2][0], ko[2][0], ko[2][1], "s2")
                            for i in (0, 1, 2):
                                q_stats(ko[i][0], ko[i][1], f"o{i}")
                            for (ol, w) in subtiles(NS, RW):
                                q_stats(N + ol, w, f"s{ol}")
                        elif idx == 6:  # after k_own h1: all own k raw
                            for i in (3, 4, 5, 6):
                                k_stats(ko[i][0], ko[i][1], f"o{i}")
                        elif idx == 7:
                            for i in (3, 4):
                                k_apply(ko[i][0], ko[i][0], ko[i][1], f"o{i}")
                            for i in (3, 4, 5, 6):
                                k_stats(N + ko[i][0], ko[i][1], f"s{i}")
                        elif idx == 8:  # after q_own h1 (moved before v)
                            for i in (5, 6):
                                k_apply(ko[i][0], ko[i][0], ko[i][1], f"o{i}")
                            for i in (3, 4):
                                k_apply(N + ko[i][0], ko[i][0], ko[i][1],
                                        f"s{i}")
                            for i in (3, 4, 5, 6):
                                q_stats(ko[i][0], ko[i][1], f"o{i}")
                        elif idx == 9:  # after v_own h1
                            for i in (5, 6):
                                k_apply(N + ko[i][0], ko[i][0], ko[i][1],
                                        f"s{i}")
                            for g in range(5):
                                v_transpose(vT, 0, g, vtp)
                        elif idx == 10:  # after v_sh h1
                            for g in range(5):
                                v_transpose(vT, 1, g, vtp)

                    wt_next = None
                    blk_idx = 0
                    for half in range(2):
                        h0 = half * 1600
                        order = ([2, 3, 4, 5, 0, 1] if half == 0
                                 else [2, 3, 0, 4, 5])
                        if half == 0:
                            wt_next = w_pool.tile([128, NCH, HD], BF16,
                                                  tag="w", name="wt_first")
                            nc.sync.dma_start(wt_next[:], w6[order[0], :, :, :])
                        xts = []
                        for c in range(NCH):
                            xt = xt_pool.tile([128, 1600], BF16,
                                              tag=f"xt{c}", name=f"xt{half}_{c}")
                            nc.sync.dma_start(
                                xt[:], xT[c * 128:(c + 1) * 128, h0:h0 + 1600])
                            xts.append(xt)
                        if half == 0:
                            # k tables + rot/eye land during h0 compute,
                            # before the first k_apply/v_transpose fillers
                            nc.sync.dma_start(tabs["ck"][:], ck[:])
                            nc.sync.dma_start(tabs["sk"][:], sk[:])
                            nc.sync.dma_start(rot_sb[:], rotm[:])
                            nc.sync.dma_start(eye_sb[:], eye[:])
                        else:
                            nc.sync.dma_start(tabs["cq"][:], cq[:])
                            nc.sync.dma_start(tabs["sq"][:], sq[:])
                            for u in range(2):
                                nc.sync.dma_start(wo_sb[u][:], wo2[u, :, :])
                        for b in order:
                            if b == 0:
                                dst, d0 = qT, h0
                            elif b == 1:
                                dst, d0 = qT, N + h0
                            elif b in (2, 3):
                                dst, d0 = kT, (b - 2) * N + h0
                            else:
                                dst, d0 = vT, (b - 4) * N + h0
                            if wt_next is not None:
                                wt_all, wt_next = wt_next, None
                            else:
                                wt_all = w_pool.tile([128, NCH, HD], BF16,
                                                     tag="w",
                                                     name=f"wt{half}_{b}")
                                nc.sync.dma_start(wt_all[:], w6[b, :, :, :])
                            tiles4 = subtiles(1600, PW)
                            pss = [pp.tile([128, PW], F32, tag="pp",
                                           name=f"pp{half}_{b}_{oi}")
                                   for oi in range(len(tiles4))]
                            for c in range(NCH):
                                for oi, (o, w) in enumerate(tiles4):
                                    nc.tensor.matmul(
                                        pss[oi][:, :w], wt_all[:, c, :],
                                        xts[c][:, o:o + w],
                                        start=(c == 0), stop=(c == NCH - 1),
                                    )
                                    if c == NCH - 1:
                                        # free the psum slot asap (pp bufs=4)
                                        nc.vector.tensor_scalar_add(
                                            dst[:, d0 + o:d0 + o + w],
                                            pss[oi][:, :w],
                                            bias_sb[:, b:b + 1],
                                        )
                            fillers_for(blk_idx)
                            blk_idx += 1

            # ---------------- attention phase ----------------
            # chunks: (unit, gco, tok, cw)
            chunks = []
            for (ol, w) in subtiles(N, CW):
                chunks.append((0, ol, ol, w))
            for (ol, w) in subtiles(NS, CW):
                chunks.append((1, N + ol, ol, w))
            NCHUNK = len(chunks)

            # psum pools: sc 2x2 + ao 2 + op 2 = 8 banks
            with tc.tile_pool(name="scp", bufs=2, space="PSUM") as scp, \
                 tc.tile_pool(name="aop", bufs=2, space="PSUM") as aop, \
                 tc.tile_pool(name="opp", bufs=2, space="PSUM") as opp, \
                 tc.tile_pool(name="expp", bufs=6) as expp, \
                 tc.tile_pool(name="accp", bufs=2) as accp, \
                 tc.tile_pool(name="qrt", bufs=2) as qrt, \
                 tc.tile_pool(name="yout", bufs=3) as yout:

                state = {}

                def rope_q(ci):
                    unit, gco, tok, cw = chunks[ci]
                    src = qT[:, gco:gco + cw]
                    rot = scp.tile([128, CW], F32, tag="sc",
                                   name=f"qrot{ci}")
                    for (off, hw) in subtiles(cw, 512):
                        nc.tensor.matmul(rot[:, off:off + hw], rot_sb[:],
                                         qT[:, gco + off:gco + off + hw],
                                         start=True, stop=True)
                    bcp = scp.tile([128, CW], F32, tag="sc",
                                   name=f"qbcp{ci}")
                    for i in range((cw + 127) // 128):
                        lo = i * 128
                        ccw = min(128, cw - lo)
                        nc.tensor.matmul(
                            bcp[:, lo:lo + ccw],
                            ones_f[0:1, :],
                            rows_all[0:1, gco + lo:gco + lo + ccw],
                            start=True, stop=True)
                    m1 = qrt.tile([128, CW], BF16, tag="m1", name=f"qm1{ci}")
                    nc.gpsimd.tensor_mul(m1[:, :cw], src,
                                         tabs["cq"][:, tok:tok + cw])
                    m2 = qrt.tile([128, CW], BF16, tag="m2", name=f"qm2{ci}")
                    nc.vector.tensor_mul(m2[:, :cw], rot[:, :cw],
                                         tabs["sq"][:, tok:tok + cw])
                    qr = qrt.tile([128, CW], BF16, tag="qr", name=f"qqr{ci}")
                    nc.gpsimd.tensor_add(qr[:, :cw], m1[:, :cw], m2[:, :cw])
                    nc.vector.tensor_mul(src, qr[:, :cw], bcp[:, :cw])

                def tail(ci):
                    """softmax denominators for chunk ci -> inv_sb cols."""
                    chain_d, chain_g = state.pop(ci)
                    unit, gco, tok, cw = chunks[ci]
                    g0 = gco // 128
                    hs = subtiles(cw, 512)
                    sms = []
                    for (off, hw) in hs:
                        sm = opp.tile([128, 512], F32, tag="op",
                                      name=f"sm{ci}_{off}")
                        nc.tensor.matmul(sm[0:1, :hw], ones_sb[:, 0:1],
                                         chain_d[:, off:off + hw],
                                         start=True, stop=False)
                        nc.tensor.matmul(sm[0:1, :hw], ones_sb[:, 0:1],
                                         chain_g[:, off:off + hw],
                                         start=False, stop=True)
                        sms.append(sm)
                    smrow = rowtmp.tile([1, CW], F32R, tag="smrow",
                                        name=f"smr{ci}")
                    for (off, hw), sm in zip(hs, sms):
                        nc.vector.tensor_copy(smrow[:, off:off + hw],
                                              sm[0:1, :hw])
                    smt = opp.tile([128, 512], F32, tag="op",
                                   name=f"smt{ci}")
                    ncc = (cw + 127) // 128
                    for i in range(ncc):
                        lo = i * 128
                        ccw = min(128, cw - lo)
                        nc.tensor.matmul(
                            smt[0:ccw, 2 * i:2 * i + 2],
                            smrow[0:1, lo:lo + ccw],
                            ones_f[0:1, 0:2],
                            start=True, stop=True)
                    nc.vector.reciprocal(inv_sb[:, g0:g0 + ncc],
                                         smt[:, 0:2 * ncc:2])

                cpat_ctr = [0]

                def outproj_tile(ci, ti, copy_eng=None):
                    """one 128-token tile of the out-projection for chunk
                    ci: 3 matmuls + 3 scaled psum->sbuf copies + dma."""
                    if copy_eng is None:
                        copy_eng = CPAT[cpat_ctr[0] % len(CPAT)]
                        cpat_ctr[0] += 1
                    unit, gco, tok, cw = chunks[ci]
                    ydst = y_own if unit == 0 else y_sh
                    it = ti * 128
                    iw = min(128, cw - it)
                    git = gco + it
                    gidx = git // 128
                    yt = yout.tile([128, D], BF16, tag="yt",
                                   name=f"yt{ci}_{ti}")
                    for ct3 in range(3):
                        op = opp.tile([128, 512], F32, tag="op",
                                      name=f"op{ci}_{ti}_{ct3}")
                        nc.tensor.matmul(
                            op[0:iw, :], aoT[:, git:git + iw],
                            wo_sb[unit][:, ct3 * 512:(ct3 + 1) * 512],
                            start=True, stop=True)
                        eng = copy_eng[ct3]
                        if eng == "d":
                            nc.vector.tensor_scalar_mul(
                                yt[0:iw, ct3 * 512:(ct3 + 1) * 512],
                                op[0:iw, :], inv_sb[0:iw, gidx:gidx + 1])
                        elif eng == "g":
                            nc.gpsimd.tensor_scalar_mul(
                                yt[0:iw, ct3 * 512:(ct3 + 1) * 512],
                                op[0:iw, :], inv_sb[0:iw, gidx:gidx + 1])
                        else:
                            nc.scalar.activation(
                                yt[0:iw, ct3 * 512:(ct3 + 1) * 512],
                                op[0:iw, :], AF.Copy,
                                scale=inv_sb[0:iw, gidx:gidx + 1])
                    nc.sync.dma_start(ydst[tok + it:tok + it + iw, :],
                                      yt[0:iw, :])

                # out-copy engines: only DVE and ACT can read PSUM;
                # ACT is exp-saturated mid-stream so the main loop uses DVE
                CPAT = ["ddd", "ddd", "dda", "ddd"]

                op_pend = []

                def attention(ci, prev_ci, next_ci):
                    """score/exp/AV stream for chunk ci. AV trails exp by
                    one k-tile so the PE never waits on ACT; the previous
                    chunk's softmax tail + pending out-projection tiles
                    and the next chunk's rope are interleaved."""
                    unit, gco, tok, cw = chunks[ci]
                    head = unit
                    hs = subtiles(cw, 512)
                    aos = [aop.tile([128, 512], F32, tag="ao",
                                    name=f"ao{ci}_{off}") for (off, hw) in hs]
                    chain_d = accp.tile([128, CW], BF16, tag="accd",
                                        name=f"accd{ci}")
                    chain_g = accp.tile([128, CW], BF16, tag="accg",
                                        name=f"accg{ci}")
                    nd = ng = 0
                    d_pend = g_pend = None
                    # out-proj consumption capacity scales with chunk width
                    cap = max(1, cw * 11 // 1024)
                    op_jts = set([3, 5, 7, 9, 11, 13, 15, 17, 19, 21, 23][:cap])
                    prev = None

                    def av(ex, jt):
                        for oi, (off, hw) in enumerate(hs):
                            nc.tensor.matmul(
                                aos[oi][:, :hw], vtok[:, head, jt, :],
                                ex[:, off:off + hw],
                                start=(jt == 0), stop=(jt == 24))

                    for jt in range(25):
                        gjt = head * 25 + jt
                        sc = scp.tile([128, CW], F32, tag="sc",
                                      name=f"sc{ci}_{jt}")
                        for (off, hw) in hs:
                            nc.tensor.matmul(
                                sc[:, off:off + hw],
                                kT[:, gjt * 128:(gjt + 1) * 128],
                                qT[:, gco + off:gco + off + hw],
                                start=True, stop=True)
                        ex = expp.tile([128, CW], BF16, tag="ex",
                                       name=f"ex{ci}_{jt}")
                        nc.scalar.activation(ex[:, :cw], sc[:, :cw], AF.Exp,
                                             scale=rsk_sb[:, gjt:gjt + 1])
                        if prev is not None:
                            av(*prev)
                        prev = (ex, jt)
                        # softmax sum chains: gpsimd carries most of
                        # the adds (DVE carries the out-proj copies)
                        if jt % 3 == 2 or jt == 24:
                            if nd == 0:
                                d_pend = ex
                            elif nd == 1:
                                nc.vector.tensor_add(chain_d[:, :cw],
                                                     d_pend[:, :cw],
                                                     ex[:, :cw])
                            else:
                                nc.vector.tensor_add(chain_d[:, :cw],
                                                     chain_d[:, :cw],
                                                     ex[:, :cw])
                            nd += 1
                        else:
                            if ng == 0:
                                g_pend = ex
                            elif ng == 1:
                                nc.gpsimd.tensor_add(chain_g[:, :cw],
                                                     g_pend[:, :cw],
                                                     ex[:, :cw])
                            else:
                                nc.gpsimd.tensor_add(chain_g[:, :cw],
                                                     chain_g[:, :cw],
                                                     ex[:, :cw])
                            ng += 1
                        if jt == 2 and prev_ci is not None:
                            tail(prev_ci)
                        if jt == 4 and next_ci is not None:
                            rope_q(next_ci)
                        if jt in op_jts and op_pend:
                            outproj_tile(*op_pend.pop(0))
                    av(*prev)
                    # copy ao psum -> aoT (frees the ao banks for ci+1)
                    for oi, (off, hw) in enumerate(hs):
                        nc.vector.tensor_copy(
                            aoT[:, gco + off:gco + off + hw],
                            aos[oi][:, :hw])
                    state[ci] = (chain_d, chain_g)
                    for ti in range((cw + 127) // 128):
                        op_pend.append((ci, ti))

                # the narrow own-head chunk goes first so it never has to
                # absorb a full chunk's out-projection work; the 576-wide
                # shared chunk sits mid-sequence so wide chunks drain its
                # backlog before the epilogue
                corder = [3, 0, 1, 5, 2, 4]
                rope_q(corder[0])
                for oi_ in range(len(corder)):
                    attention(corder[oi_],
                              corder[oi_ - 1] if oi_ > 0 else None,
                              corder[oi_ + 1] if oi_ + 1 < len(corder) else None)
                tail(corder[-1])
                # epilogue drain: exp is done, so ACT helps with copies
                EPAT = ["ada", "dad"]
                ei = 0
                while op_pend:
                    outproj_tile(*op_pend.pop(0), copy_eng=EPAT[ei % 2])
                    ei += 1

    nc.compile()
    return nc


def _get_nc():
    if "nc" not in _CACHE:
        _CACHE["nc"] = _build()
    return _CACHE["nc"]


def _host_prep(inputs):
    import ml_dtypes

    bf16 = ml_dtypes.bfloat16
    x = np.asarray(inputs["x"], np.float32)[0]          # [N, D]
    Wq = np.asarray(inputs["Wq"], np.float32)
    Wk = np.asarray(inputs["Wk"], np.float32)
    Wv = np.asarray(inputs["Wv"], np.float32)
    Wo = np.asarray(inputs["Wo"], np.float32)
    bq = np.asarray(inputs["bq"], np.float32)
    bk = np.asarray(inputs["bk"], np.float32)
    bv = np.asarray(inputs["bv"], np.float32)
    qs = np.asarray(inputs["q_scale"], np.float32)
    ks = np.asarray(inputs["k_scale"], np.float32)
    ft = np.asarray(inputs["freqs_t"], np.float32)
    fh = np.asarray(inputs["freqs_h"], np.float32)
    fw = np.asarray(inputs["freqs_w"], np.float32)

    cos = np.zeros((N, HD // 2), np.float32)
    sin = np.zeros((N, HD // 2), np.float32)
    idx = np.arange(N)
    f_idx, h_idx, w_idx = idx // (Hg * Wg), (idx // Wg) % Hg, idx % Wg
    cos[:, 0:22], sin[:, 0:22] = ft[f_idx, :, 0], ft[f_idx, :, 1]
    cos[:, 22:43], sin[:, 22:43] = fh[h_idx, :, 0], fh[h_idx, :, 1]
    cos[:, 43:64], sin[:, 43:64] = fw[w_idx, :, 0], fw[w_idx, :, 1]
    C = np.repeat(cos, 2, axis=1).T.copy()               # [128, N]
    S = np.repeat(sin, 2, axis=1).T.copy()
    qs_sw = qs.reshape(64, 2)[:, ::-1].reshape(128)
    ks_sw = ks.reshape(64, 2)[:, ::-1].reshape(128)
    Cq, Sq = C * qs[:, None], S * qs_sw[:, None]
    Ck, Sk = C * ks[:, None], S * ks_sw[:, None]

    rotm = np.zeros((128, 128), np.float32)
    pr = np.arange(64)
    rotm[2 * pr + 1, 2 * pr] = -1.0
    rotm[2 * pr, 2 * pr + 1] = 1.0
    eye = np.eye(128, dtype=np.float32)
    ones2d = np.ones((128, 128), np.float32)

    xT = np.ascontiguousarray(x.T)                       # [D, N]
    perm_swap = np.concatenate([np.arange(1600, N), np.arange(0, 1600)])

    in_maps = []
    for core in range(8):
        pair, parity = core // 2, core % 2
        own, sh = 3 * pair + parity, 3 * pair + 2
        if parity == 0:
            xTc, Cqc, Sqc, Ckc, Skc = xT, Cq, Sq, Ck, Sk
        else:
            xTc = np.ascontiguousarray(xT[:, perm_swap])
            Cqc = np.ascontiguousarray(Cq[:, perm_swap])
            Sqc = np.ascontiguousarray(Sq[:, perm_swap])
            Ckc = np.ascontiguousarray(Ck[:, perm_swap])
            Skc = np.ascontiguousarray(Sk[:, perm_swap])
        w6c = np.stack([
            Wq[:, own * HD:(own + 1) * HD], Wq[:, sh * HD:(sh + 1) * HD],
            Wk[:, own * HD:(own + 1) * HD], Wk[:, sh * HD:(sh + 1) * HD],
            Wv[:, own * HD:(own + 1) * HD], Wv[:, sh * HD:(sh + 1) * HD],
        ])
        # [6, D, HD] -> [6, 128, NCH, HD] so each block loads in ONE dma
        w6c = w6c.reshape(6, NCH, 128, HD).transpose(0, 2, 1, 3)
        bias6 = np.stack([
            bq[own * HD:(own + 1) * HD], bq[sh * HD:(sh + 1) * HD],
            bk[own * HD:(own + 1) * HD], bk[sh * HD:(sh + 1) * HD],
            bv[own * HD:(own + 1) * HD], bv[sh * HD:(sh + 1) * HD],
        ], axis=1)
        wo2 = np.stack([
            Wo[own * HD:(own + 1) * HD, :], Wo[sh * HD:(sh + 1) * HD, :],
        ])
        in_maps.append({
            "xT": xTc.astype(bf16), "w6": np.ascontiguousarray(w6c).astype(bf16),
            "bias6": np.ascontiguousarray(bias6),
            "cq": Cqc.astype(bf16), "sq": Sqc.astype(bf16),
            "ck": Ckc.astype(bf16), "sk": Skc.astype(bf16),
            "rotm": rotm.astype(bf16), "eye": eye.astype(bf16),
            "ones2d": ones2d.astype(bf16),
            "ones2df": np.ones((1, 128), np.float32),
            "wo2": np.ascontiguousarray(wo2).astype(bf16),
        })
    return in_maps, perm_swap


def _gather(results, perm_swap, bo):
    inv_swap = perm_swap  # swapping halves is its own inverse
    y = np.zeros((N, D), np.float32)
    for core in range(8):
        parity = core % 2
        yo = np.asarray(results[core]["y_own"], np.float32)
        ysh = np.asarray(results[core]["y_sh"], np.float32)
        if parity == 0:
            y += yo
            y[0:1600] += ysh
        else:
            y += yo[inv_swap]
            y[1600:3200] += ysh
    y += bo[None, :]
    return y[None]


def run_internal(inputs, trace=False, **kw):
    from concourse.bass_utils import run_bass_kernel_spmd

    nc = _get_nc()
    in_maps, perm_swap = _host_prep(inputs)
    res = run_bass_kernel_spmd(
        nc, in_maps, core_ids=list(range(8)), trace=trace, **kw
    )
    bo = np.asarray(inputs["bo"], np.float32)
    y = _gather(res.results, perm_swap, bo)
    return y, res


def kernel(**inputs):
    y, _ = run_internal(inputs, trace=False)
    return y


# revision 25
# speedup vs baseline: 1.5897x; 1.1074x over previous
"""Trainium2 Bass kernel for nn_MultiHeadAttention_61357902791348.

Sharding: 12 heads on 8 cores. Core pair (2p, 2p+1) owns heads {3p, 3p+1}
fully and splits head 3p+2's query rows (even core: rows [0,1600), odd:
[1600,3200)) -- balanced head/sequence-hybrid tensor parallelism with no
device collectives. Each core emits partial out-projection results; the
host sums the 8 partials and adds bo.

v3 rewrite vs v2 (452us -> target ~260us):
  * attention q-chunks are 1024 wide: exp runs one ACTIVATE per k-tile
    over a 2-bank PSUM score tile, amortizing the 352-cycle ACT fixed
    cost (ACT was the bottleneck engine of the attention phase)
  * projection blocks reordered K,V,Q and rope(k)/rms-stats/V-transposes
    emitted as filler between blocks: the PE stream stays dense through
    what used to be a fragmented half-clock (HAM-throttled) phase
  * q rms factors computed in row layout directly (ssq row -> sqrt ->
    reciprocal -> rows_all), deleting v2's column transpose + flatten
    machinery and its PSUM bank
  * rope(q) for chunk ci+1 is emitted inside chunk ci's score stream so
    score matmuls never wait on the DVE rope ops at a chunk boundary
  * softmax sum chains run on 1024-wide tiles split DVE/gpsimd; the two
    chains reduce into one PSUM row via two accumulating ones-matmuls
  * out-projection PSUM->SBUF copies (with the deferred 1/sum scale)
    split across DVE and gpsimd; exp keeps ACT saturated
"""

import numpy as np

B, N, D = 1, 3200, 1536
NH, HD = 12, 128
F, Hg, Wg = 8, 20, 20
EPS = 1e-6
NS = 1600          # shared-head query rows per core
NCH = D // 128     # 12 D-chunks
PW = 400           # projection moving tile width
RW = 512           # rope k tile width
CW = 1024          # attention q-chunk width
NQ = N + NS        # 4800 q tokens per core (own + shared)
NK = 2 * N         # 6400 k tokens per core (own + shared heads)

_CACHE = {}


def subtiles(total, width):
    return [(o, min(width, total - o)) for o in range(0, total, width)]


def _build():
    import concourse.bacc as bacc
    import concourse.mybir as mybir
    import concourse.tile as tile

    F32 = mybir.dt.float32
    F32R = mybir.dt.float32r
    BF16 = mybir.dt.bfloat16
    AF = mybir.ActivationFunctionType

    nc = bacc.Bacc("TRN2", target_bir_lowering=False, debug=False)

    xT = nc.dram_tensor("xT", [D, N], BF16, kind="ExternalInput")
    w6 = nc.dram_tensor("w6", [6, 128, NCH, HD], BF16, kind="ExternalInput")
    bias6 = nc.dram_tensor("bias6", [HD, 6], F32, kind="ExternalInput")
    cq = nc.dram_tensor("cq", [HD, N], BF16, kind="ExternalInput")
    sq = nc.dram_tensor("sq", [HD, N], BF16, kind="ExternalInput")
    ck = nc.dram_tensor("ck", [HD, N], BF16, kind="ExternalInput")
    sk = nc.dram_tensor("sk", [HD, N], BF16, kind="ExternalInput")
    rotm = nc.dram_tensor("rotm", [HD, HD], BF16, kind="ExternalInput")
    eye = nc.dram_tensor("eye", [HD, HD], BF16, kind="ExternalInput")
    ones2d = nc.dram_tensor("ones2d", [HD, HD], BF16, kind="ExternalInput")
    ones2df = nc.dram_tensor("ones2df", [1, HD], F32, kind="ExternalInput")
    wo2 = nc.dram_tensor("wo2", [2, HD, D], BF16, kind="ExternalInput")
    y_own = nc.dram_tensor("y_own", [N, D], BF16, kind="ExternalOutput")
    y_sh = nc.dram_tensor("y_sh", [NS, D], BF16, kind="ExternalOutput")

    with tile.TileContext(nc) as tc:
        import contextlib

        stack = contextlib.ExitStack()
        with stack:
            persist = stack.enter_context(tc.tile_pool(name="persist", bufs=1))
            qT = persist.tile([128, NQ], BF16, tag="qT")
            kT = persist.tile([128, NK], BF16, tag="kT")
            vtok = persist.tile([128, 2, 25, HD], BF16, tag="vtok")
            aoT = persist.tile([128, NQ], BF16, tag="aoT")
            bias_sb = persist.tile([HD, 6], F32, tag="bias")
            nc.sync.dma_start(bias_sb[:], bias6[:])
            ones_sb = persist.tile([HD, HD], BF16, tag="ones")
            nc.sync.dma_start(ones_sb[:], ones2d[:])
            rot_sb = persist.tile([HD, HD], BF16, tag="rot")
            eye_sb = persist.tile([HD, HD], BF16, tag="eye")
            ones_f = persist.tile([1, HD], F32, tag="onesf")
            nc.sync.dma_start(ones_f[:], ones2df[:])
            wo_sb = [persist.tile([HD, D], BF16, tag=f"wo{u}", name=f"wo_sb{u}")
                     for u in range(2)]
            tabs = {}
            for nm in ("cq", "sq", "ck", "sk"):
                tabs[nm] = persist.tile([128, N], BF16, tag=f"tab_{nm}",
                                        name=f"tab_{nm}")

            rsk_sb = persist.tile([128, 50], F32, tag="rsk")
            inv_sb = persist.tile([128, NQ // 128 + 1], F32, tag="inv")
            rows_all = persist.tile([1, NQ], F32, tag="rows_all")
            bias_q1 = persist.tile([1, 1], F32, tag="bias_q1")
            nc.vector.memset(bias_q1[:], HD * EPS)
            bias_k = persist.tile([128, 1], F32, tag="bias_k")
            nc.vector.memset(bias_k[:], EPS)

            rtmp = stack.enter_context(tc.tile_pool(name="rtmp", bufs=3))
            rowtmp = stack.enter_context(tc.tile_pool(name="rowtmp", bufs=2))
            pools = {}

            def k_stats(o, w, uid):
                """kT[:, o:o+w] raw k -> rsk_sb columns (per-token rms
                reciprocal, exp scale layout: k tokens on partitions)."""
                rowp = pools["rowp"]
                src = kT[:, o:o + w]
                q2 = rtmp.tile([128, RW], BF16, tag="q2", name=f"kq2{uid}")
                nc.gpsimd.tensor_mul(q2[:, :w], src, src)
                ssq = rowp.tile([1, RW], F32, tag="row", name=f"kssq{uid}")
                nc.tensor.matmul(ssq[:, :w], ones_sb[:, 0:1], q2[:, :w],
                                 start=True, stop=True)
                srow = rowtmp.tile([1, RW], F32, tag="srow", name=f"ksr{uid}")
                nc.scalar.activation(srow[:, :w], ssq[:, :w], AF.Copy)
                smt = rowp.tile([128, 8], F32, tag="row", name=f"ksmt{uid}")
                ncc = (w + 127) // 128
                for i in range(ncc):
                    lo = i * 128
                    ccw = min(128, w - lo)
                    nc.tensor.matmul(
                        smt[0:ccw, 2 * i:2 * i + 2],
                        srow[0:1, lo:lo + ccw],
                        ones_f[0:1, 0:2],
                        start=True, stop=True)
                sq_t = rtmp.tile([128, 8], F32, tag="sqt", name=f"ksq{uid}")
                nc.scalar.activation(sq_t[:, :ncc], smt[:, 0:2 * ncc:2],
                                     AF.Sqrt, bias=bias_k[:], scale=1.0 / HD)
                nc.vector.reciprocal(
                    rsk_sb[:, o // 128:o // 128 + ncc], sq_t[:, :ncc])

            def q_stats(o, w, uid):
                """qT[:, o:o+w] raw q -> rows_all row segment (rms recip
                with 1/sqrt(HD) folded, broadcast layout for rope)."""
                rowp = pools["rowp"]
                src = qT[:, o:o + w]
                q2 = rtmp.tile([128, RW], BF16, tag="q2", name=f"qq2{uid}")
                nc.gpsimd.tensor_mul(q2[:, :w], src, src)
                ssq = rowp.tile([1, RW], F32, tag="row", name=f"qssq{uid}")
                nc.tensor.matmul(ssq[:, :w], ones_sb[:, 0:1], q2[:, :w],
                                 start=True, stop=True)
                srow = rowtmp.tile([1, RW], F32, tag="srow", name=f"qsr{uid}")
                nc.scalar.activation(srow[:, :w], ssq[:, :w], AF.Sqrt,
                                     bias=bias_q1[:], scale=1.0)
                nc.vector.reciprocal(rows_all[0:1, o:o + w], srow[:, :w])

            def k_apply(o, tok, w, uid):
                """In-place rope on kT[:, o:o+w] (tokens tok..tok+w)."""
                src = kT[:, o:o + w]
                # rot shares the rowp pool+tag so back-to-back k_applys
                # don't serialize on a single psum slot
                rot = pools["rowp"].tile([128, RW], F32, tag="row",
                                         name=f"krt{uid}")
                nc.tensor.matmul(rot[:, :w], rot_sb[:], src,
                                 start=True, stop=True)
                m1 = rtmp.tile([128, RW], BF16, tag="m1", name=f"km1{uid}")
                nc.gpsimd.tensor_mul(m1[:, :w], src,
                                     tabs["ck"][:, tok:tok + w])
                m2 = rtmp.tile([128, RW], BF16, tag="m2", name=f"km2{uid}")
                nc.vector.tensor_mul(m2[:, :w], rot[:, :w],
                                     tabs["sk"][:, tok:tok + w])
                nc.gpsimd.tensor_add(src, m1[:, :w], m2[:, :w])

            def v_transpose(vT, h, g, vtp):
                tpg = vtp.tile([128, 5, HD], BF16, tag="tp",
                               name=f"tp{h}_{g}")
                for k5 in range(5):
                    jt = g * 5 + k5
                    nc.tensor.transpose(
                        tpg[:, k5, :],
                        vT[:, h * N + jt * 128:h * N + (jt + 1) * 128],
                        eye_sb[:])
                nc.scalar.activation(vtok[:, h, g * 5:(g + 1) * 5, :],
                                     tpg[:], AF.Copy)

            # ---------------- projection phase ----------------
            # block order per half: k_own, k_sh, v_own, v_sh, q_own(, q_sh)
            # fillers keyed by global block index, emitted after that
            # block's matmuls so rope/stats overlap later projections.
            ko = subtiles(N, RW)          # 7 tiles per head segment

            with tc.tile_pool(name="vt", bufs=1) as vt_pool:
                vT = vt_pool.tile([128, NK], BF16, tag="vT")
                with tc.tile_pool(name="xt", bufs=1) as xt_pool, \
                     tc.tile_pool(name="wld", bufs=3) as w_pool, \
                     tc.tile_pool(name="pp", bufs=4, space="PSUM") as pp, \
                     tc.tile_pool(name="rowp", bufs=2, space="PSUM") as rowp, \
                     tc.tile_pool(name="vtp", bufs=2, space="PSUM") as vtp:
                    pools["rowp"] = rowp

                    def fillers_for(idx):
                        if idx == 2:    # after v_own h0: k raw h0 ready
                            for i in (0, 1, 2):
                                k_stats(ko[i][0], ko[i][1], f"o{i}")
                        elif idx == 3:
                            for i in (0, 1, 2):
                                k_stats(N + ko[i][0], ko[i][1], f"s{i}")
                            for i in (0, 1):
                                k_apply(ko[i][0], ko[i][0], ko[i][1], f"o{i}")
                        elif idx == 4:
                            k_apply(ko[2][0], ko[2][0], ko[2][1], "o2")
                            for i in (0, 1):
                                k_apply(N + ko[i][0], ko[i][0], ko[i][1],
                                        f"s{i}")
                        elif idx == 5:
                            k_apply(N + ko[2][0], ko[2][0], ko[2][1], "s2")
                            for i in (0, 1, 2):
                                q_stats(ko[i][0], ko[i][1], f"o{i}")
                            for (ol, w) in subtiles(NS, RW):
                                q_stats(N + ol, w, f"s{ol}")
                        elif idx == 6:  # after k_own h1: all own k raw
                            for i in (3, 4, 5, 6):
                                k_stats(ko[i][0], ko[i][1], f"o{i}")
                        elif idx == 7:
                            for i in (3, 4):
                                k_apply(ko[i][0], ko[i][0], ko[i][1], f"o{i}")
                            for i in (3, 4, 5, 6):
                                k_stats(N + ko[i][0], ko[i][1], f"s{i}")
                        elif idx == 8:  # after q_own h1 (moved before v)
                            for i in (5, 6):
                                k_apply(ko[i][0], ko[i][0], ko[i][1], f"o{i}")
                            for i in (3, 4):
                                k_apply(N + ko[i][0], ko[i][0], ko[i][1],
                                        f"s{i}")
                            for i in (3, 4, 5, 6):
                                q_stats(ko[i][0], ko[i][1], f"o{i}")
                        elif idx == 9:  # after v_own h1
                            for i in (5, 6):
                                k_apply(N + ko[i][0], ko[i][0], ko[i][1],
                                        f"s{i}")
                            for g in range(5):
                                v_transpose(vT, 0, g, vtp)
                        elif idx == 10:  # after v_sh h1
                            for g in range(5):
                                v_transpose(vT, 1, g, vtp)

                    wt_next = None
                    blk_idx = 0
                    for half in range(2):
                        h0 = half * 1600
                        order = ([2, 3, 4, 5, 0, 1] if half == 0
                                 else [2, 3, 0, 4, 5])
                        if half == 0:
                            wt_next = w_pool.tile([128, NCH, HD], BF16,
                                                  tag="w", name="wt_first")
                            nc.sync.dma_start(wt_next[:], w6[order[0], :, :, :])
                        xts = []
                        for c in range(NCH):
                            xt = xt_pool.tile([128, 1600], BF16,
                                              tag=f"xt{c}", name=f"xt{half}_{c}")
                            nc.sync.dma_start(
                                xt[:], xT[c * 128:(c + 1) * 128, h0:h0 + 1600])
                            xts.append(xt)
                        if half == 0:
                            # k tables + rot/eye land during h0 compute,
                            # before the first k_apply/v_transpose fillers
                            nc.sync.dma_start(tabs["ck"][:], ck[:])
                            nc.sync.dma_start(tabs["sk"][:], sk[:])
                            nc.sync.dma_start(rot_sb[:], rotm[:])
                            nc.sync.dma_start(eye_sb[:], eye[:])
                        else:
                            nc.sync.dma_start(tabs["cq"][:], cq[:])
                            nc.sync.dma_start(tabs["sq"][:], sq[:])
                            for u in range(2):
                                nc.sync.dma_start(wo_sb[u][:], wo2[u, :, :])
                        for b in order:
                            if b == 0:
                                dst, d0 = qT, h0
                            elif b == 1:
                                dst, d0 = qT, N + h0
                            elif b in (2, 3):
                                dst, d0 = kT, (b - 2) * N + h0
                            else:
                                dst, d0 = vT, (b - 4) * N + h0
                            if wt_next is not None:
                                wt_all, wt_next = wt_next, None
                            else:
                                wt_all = w_pool.tile([128, NCH, HD], BF16,
                                                     tag="w",
                                                     name=f"wt{half}_{b}")
                                nc.sync.dma_start(wt_all[:], w6[b, :, :, :])
                            tiles4 = subtiles(1600, PW)
                            pss = [pp.tile([128, PW], F32, tag="pp",
                                           name=f"pp{half}_{b}_{oi}")
                                   for oi in range(len(tiles4))]
                            for c in range(NCH):
                                for oi, (o, w) in enumerate(tiles4):
                                    nc.tensor.matmul(
                                        pss[oi][:, :w], wt_all[:, c, :],
                                        xts[c][:, o:o + w],
                                        start=(c == 0), stop=(c == NCH - 1),
                                    )
                                    if c == NCH - 1:
                                        # free the psum slot asap (pp bufs=4)
                                        nc.vector.tensor_scalar_add(
                                            dst[:, d0 + o:d0 + o + w],
                                            pss[oi][:, :w],
                                            bias_sb[:, b:b + 1],
                                        )
                            fillers_for(blk_idx)
                            blk_idx += 1

            # ---------------- attention phase ----------------
            # chunks: (unit, gco, tok, cw)
            chunks = []
            for (ol, w) in subtiles(N, CW):
                chunks.append((0, ol, ol, w))
            for (ol, w) in subtiles(NS, CW):
                chunks.append((1, N + ol, ol, w))
            NCHUNK = len(chunks)

            # psum pools: sc 2x2 + ao 2 + op 2 = 8 banks
            with tc.tile_pool(name="scp", bufs=2, space="PSUM") as scp, \
                 tc.tile_pool(name="aop", bufs=2, space="PSUM") as aop, \
                 tc.tile_pool(name="opp", bufs=1, space="PSUM") as opp, \
                 tc.tile_pool(name="smp", bufs=1, space="PSUM") as smp, \
                 tc.tile_pool(name="expp", bufs=8) as expp, \
                 tc.tile_pool(name="qrt", bufs=2) as qrt, \
                 tc.tile_pool(name="yout", bufs=3) as yout:

                state = {}

                def rope_q(ci):
                    unit, gco, tok, cw = chunks[ci]
                    src = qT[:, gco:gco + cw]
                    rot = scp.tile([128, CW], F32, tag="sc",
                                   name=f"qrot{ci}")
                    for (off, hw) in subtiles(cw, 512):
                        nc.tensor.matmul(rot[:, off:off + hw], rot_sb[:],
                                         qT[:, gco + off:gco + off + hw],
                                         start=True, stop=True)
                    bcp = scp.tile([128, CW], F32, tag="sc",
                                   name=f"qbcp{ci}")
                    for i in range((cw + 127) // 128):
                        lo = i * 128
                        ccw = min(128, cw - lo)
                        nc.tensor.matmul(
                            bcp[:, lo:lo + ccw],
                            ones_f[0:1, :],
                            rows_all[0:1, gco + lo:gco + lo + ccw],
                            start=True, stop=True)
                    m1 = qrt.tile([128, CW], BF16, tag="m1", name=f"qm1{ci}")
                    nc.gpsimd.tensor_mul(m1[:, :cw], src,
                                         tabs["cq"][:, tok:tok + cw])
                    m2 = qrt.tile([128, CW], BF16, tag="m2", name=f"qm2{ci}")
                    nc.vector.tensor_mul(m2[:, :cw], rot[:, :cw],
                                         tabs["sq"][:, tok:tok + cw])
                    qr = qrt.tile([128, CW], BF16, tag="qr", name=f"qqr{ci}")
                    nc.gpsimd.tensor_add(qr[:, :cw], m1[:, :cw], m2[:, :cw])
                    nc.vector.tensor_mul(src, qr[:, :cw], bcp[:, :cw])

                def tail(ci):
                    """softmax denominators for chunk ci -> inv_sb cols.
                    The per-half sums were accumulated on the PE into rows
                    0 / 32 of the chunk's 1-bank sum tile."""
                    sum_ps = state.pop(ci)
                    unit, gco, tok, cw = chunks[ci]
                    g0 = gco // 128
                    hs = subtiles(cw, 512)
                    smrow = rowtmp.tile([1, CW], F32, tag="smrow",
                                        name=f"smr{ci}")
                    for hi, (off, hw) in enumerate(hs):
                        nc.vector.tensor_copy(
                            smrow[:, off:off + hw],
                            sum_ps[32 * hi:32 * hi + 1, :hw])
                    smt = opp.tile([128, 512], F32, tag="op",
                                   name=f"smt{ci}")
                    ncc = (cw + 127) // 128
                    for i in range(ncc):
                        lo = i * 128
                        ccw = min(128, cw - lo)
                        nc.tensor.matmul(
                            smt[0:ccw, 2 * i:2 * i + 2],
                            smrow[0:1, lo:lo + ccw],
                            ones_f[0:1, 0:2],
                            start=True, stop=True)
                    nc.vector.reciprocal(inv_sb[:, g0:g0 + ncc],
                                         smt[:, 0:2 * ncc:2])

                cpat_ctr = [0]

                def outproj_tile(ci, ti, copy_eng=None, pool=None):
                    """one 128-token tile of the out-projection for chunk
                    ci: 3 matmuls + 3 scaled psum->sbuf copies + dma."""
                    if copy_eng is None:
                        copy_eng = CPAT[cpat_ctr[0] % len(CPAT)]
                        cpat_ctr[0] += 1
                    unit, gco, tok, cw = chunks[ci]
                    ydst = y_own if unit == 0 else y_sh
                    it = ti * 128
                    iw = min(128, cw - it)
                    git = gco + it
                    gidx = git // 128
                    yt = yout.tile([128, D], BF16, tag="yt",
                                   name=f"yt{ci}_{ti}")
                    for ct3 in range(3):
                        if pool is None:
                            op = opp.tile([128, 512], F32, tag="op",
                                          name=f"op{ci}_{ti}_{ct3}")
                        else:
                            opw = pool.tile([128, CW], F32, tag="sc",
                                            name=f"epo{ci}_{ti}_{ct3}")
                            op = opw[:, 0:512]
                        nc.tensor.matmul(
                            op[0:iw, :], aoT[:, git:git + iw],
                            wo_sb[unit][:, ct3 * 512:(ct3 + 1) * 512],
                            start=True, stop=True)
                        eng = copy_eng[ct3]
                        if eng == "d":
                            nc.vector.tensor_scalar_mul(
                                yt[0:iw, ct3 * 512:(ct3 + 1) * 512],
                                op[0:iw, :], inv_sb[0:iw, gidx:gidx + 1])
                        elif eng == "g":
                            nc.gpsimd.tensor_scalar_mul(
                                yt[0:iw, ct3 * 512:(ct3 + 1) * 512],
                                op[0:iw, :], inv_sb[0:iw, gidx:gidx + 1])
                        else:
                            nc.scalar.activation(
                                yt[0:iw, ct3 * 512:(ct3 + 1) * 512],
                                op[0:iw, :], AF.Copy,
                                scale=inv_sb[0:iw, gidx:gidx + 1])
                    nc.sync.dma_start(ydst[tok + it:tok + it + iw, :],
                                      yt[0:iw, :])

                # out-copy engines: only DVE and ACT can read PSUM;
                # ACT is exp-saturated mid-stream so the main loop uses DVE
                CPAT = ["ddd"]

                op_pend = []

                def attention(ci, prev_ci, next_ci):
                    """score/exp/AV stream for chunk ci. AV trails exp by
                    one k-tile so the PE never waits on ACT; the previous
                    chunk's softmax tail + pending out-projection tiles
                    and the next chunk's rope are interleaved."""
                    unit, gco, tok, cw = chunks[ci]
                    head = unit
                    hs = subtiles(cw, 512)
                    aos = [aop.tile([128, 512], F32, tag="ao",
                                    name=f"ao{ci}_{off}") for (off, hw) in hs]
                    sum_ps = smp.tile([64, 512], F32, tag="sm",
                                      name=f"sum{ci}")
                    # out-proj consumption capacity scales with chunk width
                    cap = max(1, cw * 11 // 1024)
                    op_jts = set([3, 5, 7, 9, 11, 13, 15, 17, 19, 21, 23][:cap])
                    prev = None
                    if prev_ci is not None:
                        tail(prev_ci)

                    def av(ex, jt):
                        for oi, (off, hw) in enumerate(hs):
                            nc.tensor.matmul(
                                aos[oi][:, :hw], vtok[:, head, jt, :],
                                ex[:, off:off + hw],
                                start=(jt == 0), stop=(jt == 24))
                        # softmax sums: accumulating ones-matmuls on the
                        # PE into the chunk's sum rows (0 and 32)
                        for hi, (off, hw) in enumerate(hs):
                            nc.tensor.matmul(
                                sum_ps[32 * hi:32 * hi + 1, :hw],
                                ones_sb[:, 0:1], ex[:, off:off + hw],
                                start=(jt == 0), stop=(jt == 24),
                                skip_group_check=True)

                    for jt in range(25):
                        gjt = head * 25 + jt
                        sc = scp.tile([128, CW], F32, tag="sc",
                                      name=f"sc{ci}_{jt}")
                        for (off, hw) in hs:
                            nc.tensor.matmul(
                                sc[:, off:off + hw],
                                kT[:, gjt * 128:(gjt + 1) * 128],
                                qT[:, gco + off:gco + off + hw],
                                start=True, stop=True)
                        ex = expp.tile([128, CW], BF16, tag="ex",
                                       name=f"ex{ci}_{jt}")
                        nc.scalar.activation(ex[:, :cw], sc[:, :cw], AF.Exp,
                                             scale=rsk_sb[:, gjt:gjt + 1])
                        if prev is not None:
                            av(*prev)
                        prev = (ex, jt)
                        if jt == 4 and next_ci is not None:
                            rope_q(next_ci)
                        if jt in op_jts and op_pend:
                            outproj_tile(*op_pend.pop(0))
                    av(*prev)
                    # copy ao psum -> aoT (frees the ao banks for ci+1)
                    for oi, (off, hw) in enumerate(hs):
                        nc.vector.tensor_copy(
                            aoT[:, gco + off:gco + off + hw],
                            aos[oi][:, :hw])
                    state[ci] = sum_ps
                    for ti in range((cw + 127) // 128):
                        op_pend.append((ci, ti))

                # the narrow own-head chunk goes first so it never has to
                # absorb a full chunk's out-projection work; the 576-wide
                # shared chunk sits mid-sequence so wide chunks drain its
                # backlog before the epilogue
                corder = [3, 0, 1, 5, 2, 4]
                rope_q(corder[0])
                for oi_ in range(len(corder)):
                    attention(corder[oi_],
                              corder[oi_ - 1] if oi_ > 0 else None,
                              corder[oi_ + 1] if oi_ + 1 < len(corder) else None)
                tail(corder[-1])
                # epilogue drain: exp is done, so ACT helps with copies
                EPAT = ["ada", "dad"]
                ei = 0
                while op_pend:
                    outproj_tile(*op_pend.pop(0), copy_eng=EPAT[ei % 2],
                                 pool=scp)
                    ei += 1

    nc.compile()
    return nc


def _get_nc():
    if "nc" not in _CACHE:
        _CACHE["nc"] = _build()
    return _CACHE["nc"]


def _host_prep(inputs):
    import ml_dtypes

    bf16 = ml_dtypes.bfloat16
    x = np.asarray(inputs["x"], np.float32)[0]          # [N, D]
    Wq = np.asarray(inputs["Wq"], np.float32)
    Wk = np.asarray(inputs["Wk"], np.float32)
    Wv = np.asarray(inputs["Wv"], np.float32)
    Wo = np.asarray(inputs["Wo"], np.float32)
    bq = np.asarray(inputs["bq"], np.float32)
    bk = np.asarray(inputs["bk"], np.float32)
    bv = np.asarray(inputs["bv"], np.float32)
    qs = np.asarray(inputs["q_scale"], np.float32)
    ks = np.asarray(inputs["k_scale"], np.float32)
    ft = np.asarray(inputs["freqs_t"], np.float32)
    fh = np.asarray(inputs["freqs_h"], np.float32)
    fw = np.asarray(inputs["freqs_w"], np.float32)

    cos = np.zeros((N, HD // 2), np.float32)
    sin = np.zeros((N, HD // 2), np.float32)
    idx = np.arange(N)
    f_idx, h_idx, w_idx = idx // (Hg * Wg), (idx // Wg) % Hg, idx % Wg
    cos[:, 0:22], sin[:, 0:22] = ft[f_idx, :, 0], ft[f_idx, :, 1]
    cos[:, 22:43], sin[:, 22:43] = fh[h_idx, :, 0], fh[h_idx, :, 1]
    cos[:, 43:64], sin[:, 43:64] = fw[w_idx, :, 0], fw[w_idx, :, 1]
    C = np.repeat(cos, 2, axis=1).T.copy()               # [128, N]
    S = np.repeat(sin, 2, axis=1).T.copy()
    qs_sw = qs.reshape(64, 2)[:, ::-1].reshape(128)
    ks_sw = ks.reshape(64, 2)[:, ::-1].reshape(128)
    Cq, Sq = C * qs[:, None], S * qs_sw[:, None]
    Ck, Sk = C * ks[:, None], S * ks_sw[:, None]

    rotm = np.zeros((128, 128), np.float32)
    pr = np.arange(64)
    rotm[2 * pr + 1, 2 * pr] = -1.0
    rotm[2 * pr, 2 * pr + 1] = 1.0
    eye = np.eye(128, dtype=np.float32)
    ones2d = np.ones((128, 128), np.float32)

    xT = np.ascontiguousarray(x.T)                       # [D, N]
    perm_swap = np.concatenate([np.arange(1600, N), np.arange(0, 1600)])

    in_maps = []
    for core in range(8):
        pair, parity = core // 2, core % 2
        own, sh = 3 * pair + parity, 3 * pair + 2
        if parity == 0:
            xTc, Cqc, Sqc, Ckc, Skc = xT, Cq, Sq, Ck, Sk
        else:
            xTc = np.ascontiguousarray(xT[:, perm_swap])
            Cqc = np.ascontiguousarray(Cq[:, perm_swap])
            Sqc = np.ascontiguousarray(Sq[:, perm_swap])
            Ckc = np.ascontiguousarray(Ck[:, perm_swap])
            Skc = np.ascontiguousarray(Sk[:, perm_swap])
        w6c = np.stack([
            Wq[:, own * HD:(own + 1) * HD], Wq[:, sh * HD:(sh + 1) * HD],
            Wk[:, own * HD:(own + 1) * HD], Wk[:, sh * HD:(sh + 1) * HD],
            Wv[:, own * HD:(own + 1) * HD], Wv[:, sh * HD:(sh + 1) * HD],
        ])
        # [6, D, HD] -> [6, 128, NCH, HD] so each block loads in ONE dma
        w6c = w6c.reshape(6, NCH, 128, HD).transpose(0, 2, 1, 3)
        bias6 = np.stack([
            bq[own * HD:(own + 1) * HD], bq[sh * HD:(sh + 1) * HD],
            bk[own * HD:(own + 1) * HD], bk[sh * HD:(sh + 1) * HD],
            bv[own * HD:(own + 1) * HD], bv[sh * HD:(sh + 1) * HD],
        ], axis=1)
        wo2 = np.stack([
            Wo[own * HD:(own + 1) * HD, :], Wo[sh * HD:(sh + 1) * HD, :],
        ])
        in_maps.append({
            "xT": xTc.astype(bf16), "w6": np.ascontiguousarray(w6c).astype(bf16),
            "bias6": np.ascontiguousarray(bias6),
            "cq": Cqc.astype(bf16), "sq": Sqc.astype(bf16),
            "ck": Ckc.astype(bf16), "sk": Skc.astype(bf16),
            "rotm": rotm.astype(bf16), "eye": eye.astype(bf16),
            "ones2d": ones2d.astype(bf16),
            "ones2df": np.ones((1, 128), np.float32),
            "wo2": np.ascontiguousarray(wo2).astype(bf16),
        })
    return in_maps, perm_swap


def _gather(results, perm_swap, bo):
    inv_swap = perm_swap  # swapping halves is its own inverse
    y = np.zeros((N, D), np.float32)
    for core in range(8):
        parity = core % 2
        yo = np.asarray(results[core]["y_own"], np.float32)
        ysh = np.asarray(results[core]["y_sh"], np.float32)
        if parity == 0:
            y += yo
            y[0:1600] += ysh
        else:
            y += yo[inv_swap]
            y[1600:3200] += ysh
    y += bo[None, :]
    return y[None]


def run_internal(inputs, trace=False, **kw):
    from concourse.bass_utils import run_bass_kernel_spmd

    nc = _get_nc()
    in_maps, perm_swap = _host_prep(inputs)
    res = run_bass_kernel_spmd(
        nc, in_maps, core_ids=list(range(8)), trace=trace, **kw
    )
    bo = np.asarray(inputs["bo"], np.float32)
    y = _gather(res.results, perm_swap, bo)
    return y, res


def kernel(**inputs):
    y, _ = run_internal(inputs, trace=False)
    return y
